# revision 4
# baseline (speedup 1.0000x reference)
"""Trainium2 Bass kernel for nn_Block_30107720745811 (dense transformer block).

B=4, S=1024, H=1024, NH=16. 8 NeuronCores, zero-communication sharding:
core c computes batch b=c//2, query rows (c%2)*512:(c%2)*512+512.
K/V projections are duplicated within each batch pair (no collectives).

All activations live transposed [feature, token] in SBUF; weights stream
from HBM in natural [in, out] layout as matmul stationary operands.
Matmuls run in float32r (full PE rate for N>=256, ~tf32 precision).
"""
import numpy as np
import concourse.bass as bass
import concourse.tile as tile
from concourse import mybir
from concourse import bass_utils
from concourse.alu_op_type import AluOpType as OP

AF = mybir.ActivationFunctionType
F32 = mybir.dt.float32
F32R = mybir.dt.float32r

B, S, H, NH = 4, 1024, 1024, 16
D = H // NH          # 64
P = 128
T = 512              # query tokens per core
KC = H // P          # 8 feature chunks
FC = 4 * H // P      # 32 ffn hidden chunks
HPC = P // D         # heads per feature chunk = 2
INF = 1e10
EPS = 1e-5
SCALE = 8.0          # sqrt(D)

# vec tensor column map
C_SBQ, C_SBK, C_SBO = 0, 8, 16
C_CBQ, C_CBK, C_CBO = 24, 32, 40
C_SAB, C_CAB = 48, 56
C_G, C_B = 64, 72
C_B1, C_B2, C_EPS = 80, 112, 120
NVEC = 121

MAX_WAITS = 1


def _legalize_waits(nc, max_waits=MAX_WAITS):
    """Split >max_waits semaphore waits into preceding same-engine NOPs
    (this walrus build allows only one sync wait per instruction)."""
    n_split = 0
    for f in nc.m.functions:
        for blk in f.blocks:
            out = []
            for ins in blk.instructions:
                si = getattr(ins, "sync_info", None)
                if si is not None and si.on_wait and len(si.on_wait) > max_waits:
                    waits = list(si.on_wait)
                    extra, keep = waits[:-max_waits], waits[-max_waits:]
                    for j in range(0, len(extra), max_waits):
                        out.append(mybir.InstNoOp(
                            name=f"{ins.name}-lw{j}",
                            engine=ins.engine,
                            sync_info=mybir.SyncInfo(
                                on_wait=extra[j:j + max_waits], on_update=[]),
                            bass_nofuse=True,
                        ))
                    ins.sync_info = mybir.SyncInfo(
                        on_wait=keep, on_update=list(si.on_update))
                    n_split += 1
                out.append(ins)
            blk.instructions = out
    return n_split


def _build(dbg=False):
    nc = bass.Bass("TRN2", target_bir_lowering=False, debug=False)

    def din(name, shape, dt=F32R):
        return nc.dram_tensor(name, shape, dt, kind="ExternalInput").ap()

    xq_d = din("xqT", [H, T])            # query-side hidden, transposed
    xk_d = din("xkT", [H, S])            # full hidden (self K/V), transposed
    xc_d = din("xcT", [H, S])            # full cross hidden, transposed
    w_names = ["sWq", "sWk", "sWv", "sWo", "cWq", "cWk", "cWv", "cWo"]
    w_d = {n: din(n, [H, H]) for n in w_names}
    w1_d = din("W1", [H, 4 * H])
    w2_d = din("W2", [4 * H, H])
    vec_d = din("vec", [P, NVEC], F32)
    ones_d = din("ones", [P, 1])
    onesr_d = din("onesr", [1, P])
    out_d = nc.dram_tensor("out", [H, T], F32, kind="ExternalOutput").ap()
    dbg_d = {}
    if dbg:
        for n, shape in [("d_qT", [H, T]), ("d_kT", [H, S]), ("d_v", [S, H]),
                         ("d_at", [H, T]), ("d_sa", [H, T]), ("d_snn", [H, T]),
                         ("d_h", [H, T]), ("d_u", [4 * H, T])]:
            dbg_d[n] = nc.dram_tensor(n, shape, F32, kind="ExternalOutput").ap()

    with (
        tile.TileContext(nc) as tc,
        nc.allow_low_precision(reason="fp32r activations feed matmuls"),
        tc.tile_pool(name="glob", bufs=1) as glob,
        tc.tile_pool(name="ps", bufs=1, space="PSUM") as ps,
    ):
        # ---- constants / vectors ----
        vec = glob.tile([P, NVEC], F32, tag="vec")
        nc.sync.dma_start(vec[:], vec_d[:])
        ones = glob.tile([P, 1], F32R, tag="ones")
        nc.sync.dma_start(ones[:], ones_d[:])
        onesr = glob.tile([1, P], F32R, tag="onesr")
        nc.sync.dma_start(onesr[:], onesr_d[:])
        xq = glob.tile([P, KC, T], F32R, tag="xq")
        nc.sync.dma_start(xq[:], xq_d.rearrange("(c p) t -> p c t", p=P))

        def ln(lnp, src, gcol, bcol, dbg_name=None):
            """LayerNorm over the partition-chunk axis of src [P, KC, T];
            returns a new f32r tile (glob tag 'lnq'). lnp: scratch pool."""
            psS = ps.tile([1, T], F32, tag="d", bufs=2)
            psQ = ps.tile([1, T], F32, tag="d", bufs=2)
            for m in range(KC):
                nc.tensor.matmul(psS[:], ones[:], src[:, m, :],
                                 start=(m == 0), stop=(m == KC - 1),
                                 skip_group_check=True)
                sq = lnp.tile([P, T], F32R, tag="sq", bufs=2)
                nc.scalar.activation(sq[:], src[:, m, :], AF.Square)
                nc.tensor.matmul(psQ[:], ones[:], sq[:],
                                 start=(m == 0), stop=(m == KC - 1),
                                 skip_group_check=True)
            mean = lnp.tile([1, T], F32, tag="lnv", bufs=4)
            nc.scalar.mul(mean[:], psS[:], 1.0 / H)
            ex2 = lnp.tile([1, T], F32, tag="lnv", bufs=4)
            nc.scalar.mul(ex2[:], psQ[:], 1.0 / H)
            var = lnp.tile([1, T], F32, tag="lnv", bufs=4)
            nc.vector.tensor_tensor(var[:], mean[:], mean[:], op=OP.mult)
            nc.vector.tensor_tensor(var[:], ex2[:], var[:], op=OP.subtract)
            std = lnp.tile([1, T], F32, tag="lnv", bufs=4)
            nc.scalar.activation(std[:], var[:], AF.Sqrt,
                                 bias=vec[0:1, C_EPS:C_EPS + 1])
            rstd = lnp.tile([1, T], F32R, tag="lnr", bufs=2)
            nc.vector.reciprocal(rstd[:], std[:])
            nm = lnp.tile([1, T], F32R, tag="lnr", bufs=2)
            nc.vector.tensor_tensor(nm[:], mean[:], rstd.bitcast(F32)[:],
                                    op=OP.mult)
            nc.vector.tensor_scalar_mul(nm[:], nm[:], -1.0)
            psA = ps.tile([P, T], F32, tag="ss", bufs=2)
            nc.tensor.matmul(psA[:], onesr[:], rstd[:], start=True, stop=True)
            psC = ps.tile([P, T], F32, tag="ss", bufs=2)
            nc.tensor.matmul(psC[:], onesr[:], nm[:], start=True, stop=True)
            dst = glob.tile([P, KC, T], F32R, tag="lnq")
            for m in range(KC):
                t1 = lnp.tile([P, T], F32, tag="lnt", bufs=2)
                nc.vector.scalar_tensor_tensor(t1[:], src.bitcast(F32)[:, m, :],
                                               0.0, psA[:], op0=OP.bypass,
                                               op1=OP.mult)
                nc.vector.scalar_tensor_tensor(t1[:], t1[:], 0.0, psC[:],
                                               op0=OP.bypass, op1=OP.add)
                nc.scalar.activation(dst[:, m, :], t1[:], AF.Identity,
                                     bias=vec[:, bcol + m:bcol + m + 1],
                                     scale=vec[:, gcol + m:gcol + m + 1])
            if dbg and dbg_name:
                nc.sync.dma_start(
                    dbg_d[dbg_name].rearrange("(c p) t -> p c t", p=P),
                    dst.bitcast(F32)[:])
            return dst

        def attention(pool, q_src, x_kv, Wq, Wk, Wv, Wo, qb_col, kb_col,
                      ob_col, ab_col, dbg_prefix=None):
            """Full MHA incl. out-proj + residual(xq): returns sa [P, KC, T]
            f32r (glob tag 'res')."""
            # V projection, natural [token, feature] layout
            vt = pool.tile([P, KC, H], F32R, tag="vt")
            NS = H // 4  # 256
            for n in range(4):
                wv = pool.tile([P, KC, NS], F32R, tag="wmov", bufs=2)
                nc.sync.dma_start(
                    wv[:], Wv.rearrange("(c p) n -> p c n", p=P)
                    [:, :, n * NS:(n + 1) * NS])
                for i in range(KC):
                    pv = ps.tile([P, NS], F32, tag="mm", bufs=2)
                    for k in range(KC):
                        nc.tensor.matmul(pv[:],
                                         x_kv[:, k, i * P:(i + 1) * P],
                                         wv[:, k, :],
                                         start=(k == 0), stop=(k == KC - 1))
                    nc.vector.tensor_copy(vt[:, i, n * NS:(n + 1) * NS], pv[:])
            if dbg and dbg_prefix == "s":
                nc.sync.dma_start(
                    dbg_d["d_v"].rearrange("(c p) f -> p c f", p=P),
                    vt.bitcast(F32)[:])

            at = pool.tile([P, KC, T], F32R, tag="at")
            for m in range(KC):
                # Q projection chunk m -> qt [P, T]
                wq = pool.tile([P, KC, P], F32R, tag="wst", bufs=2)
                nc.sync.dma_start(
                    wq[:], Wq.rearrange("(c p) m -> p c m", p=P)
                    [:, :, m * P:(m + 1) * P])
                pq = ps.tile([P, T], F32, tag="mm", bufs=2)
                for k in range(KC):
                    nc.tensor.matmul(pq[:], wq[:, k, :], q_src[:, k, :],
                                     start=(k == 0), stop=(k == KC - 1))
                qt = pool.tile([P, T], F32R, tag="qt", bufs=2)
                nc.scalar.activation(qt[:], pq[:], AF.Identity,
                                     bias=vec[:, qb_col + m:qb_col + m + 1])
                if dbg and dbg_prefix == "s":
                    nc.sync.dma_start(dbg_d["d_qT"][m * P:(m + 1) * P, :],
                                      qt.bitcast(F32)[:])
                # K projection chunk m -> kt [P, S]
                wk = pool.tile([P, KC, P], F32R, tag="wst", bufs=2)
                nc.sync.dma_start(
                    wk[:], Wk.rearrange("(c p) m -> p c m", p=P)
                    [:, :, m * P:(m + 1) * P])
                kt = pool.tile([P, S], F32R, tag="kt", bufs=2)
                for n in range(2):
                    pk = ps.tile([P, T], F32, tag="mm", bufs=2)
                    for k in range(KC):
                        nc.tensor.matmul(pk[:], wk[:, k, :],
                                         x_kv[:, k, n * T:(n + 1) * T],
                                         start=(k == 0), stop=(k == KC - 1))
                    nc.scalar.activation(kt[:, n * T:(n + 1) * T], pk[:],
                                         AF.Identity,
                                         bias=vec[:, kb_col + m:kb_col + m + 1])
                if dbg and dbg_prefix == "s":
                    nc.sync.dma_start(dbg_d["d_kT"][m * P:(m + 1) * P, :],
                                      kt.bitcast(F32)[:])
                # the two heads of chunk m
                for h2 in range(HPC):
                    h = HPC * m + h2
                    hb = h2 * D
                    psAv = ps.tile([P, T], F32, tag="av", bufs=2)
                    psDe = ps.tile([1, T], F32, tag="d", bufs=2)
                    for i in range(KC):
                        pss = ps.tile([P, T], F32, tag="ss", bufs=2)
                        nc.tensor.matmul(pss[:],
                                         kt[hb:hb + D, i * P:(i + 1) * P],
                                         qt[hb:hb + D, :],
                                         start=True, stop=True)
                        et = pool.tile([P, T], F32R, tag="exp", bufs=2)
                        nc.scalar.activation(
                            et[:], pss[:], AF.Exp,
                            bias=vec[:, ab_col + i:ab_col + i + 1],
                            scale=1.0 / (SCALE * SCALE))
                        nc.tensor.matmul(psDe[:], ones[:], et[:],
                                         start=(i == 0), stop=(i == KC - 1),
                                         skip_group_check=True)
                        nc.tensor.matmul(psAv[0:D, :],
                                         vt[:, i, h * D:(h + 1) * D], et[:],
                                         start=(i == 0), stop=(i == KC - 1),
                                         skip_group_check=True)
                    rc = pool.tile([1, T], F32R, tag="rc", bufs=2)
                    nc.vector.reciprocal(rc[:], psDe[:])
                    psB = ps.tile([P, T], F32, tag="av", bufs=2)
                    nc.tensor.matmul(psB[:], onesr[:], rc[:],
                                     start=True, stop=True)
                    rb = pool.tile([D, T], F32, tag="rb", bufs=2)
                    nc.scalar.copy(rb[:], psB[0:D, :])
                    if h2 == 0:
                        nc.vector.tensor_tensor(at[0:D, m, :], psAv[0:D, :],
                                                rb[:], op=OP.mult)
                    else:
                        atmp = pool.tile([D, T], F32R, tag="atmp", bufs=2)
                        nc.vector.tensor_tensor(atmp[:], psAv[0:D, :],
                                                rb[:], op=OP.mult)
                        nc.sync.dma_start(at[D:P, m, :], atmp[:])
            if dbg and dbg_prefix == "s":
                nc.sync.dma_start(
                    dbg_d["d_at"].rearrange("(c p) t -> p c t", p=P),
                    at.bitcast(F32)[:])

            # out projection + bias' + residual (original xq)
            sa = glob.tile([P, KC, T], F32R, tag="res")
            for m in range(KC):
                wo = pool.tile([P, KC, P], F32R, tag="wst", bufs=2)
                nc.sync.dma_start(
                    wo[:], Wo.rearrange("(c p) m -> p c m", p=P)
                    [:, :, m * P:(m + 1) * P])
                po = ps.tile([P, T], F32, tag="mm", bufs=2)
                for k in range(KC):
                    nc.tensor.matmul(po[:], wo[:, k, :], at[:, k, :],
                                     start=(k == 0), stop=(k == KC - 1))
                nc.vector.scalar_tensor_tensor(
                    sa[:, m, :], po[:], vec[:, ob_col + m:ob_col + m + 1],
                    xq.bitcast(F32)[:, m, :], op0=OP.add, op1=OP.add)
            return sa

        # ================= self attention =================
        with tc.tile_pool(name="attn_s", bufs=1) as pool:
            xk = pool.tile([P, KC, S], F32R, tag="xfull")
            nc.sync.dma_start(xk[:], xk_d.rearrange("(c p) t -> p c t", p=P))
            sa = attention(pool, xq, xk, w_d["sWq"], w_d["sWk"], w_d["sWv"],
                           w_d["sWo"], C_SBQ, C_SBK, C_SBO, C_SAB,
                           dbg_prefix="s")
        if dbg:
            nc.sync.dma_start(
                dbg_d["d_sa"].rearrange("(c p) t -> p c t", p=P),
                sa.bitcast(F32)[:])
        with tc.tile_pool(name="ln1", bufs=1) as lnp:
            snn = ln(lnp, sa, C_G, C_B, dbg_name="d_snn")

        # ================= cross attention =================
        with tc.tile_pool(name="attn_c", bufs=1) as pool:
            xc = pool.tile([P, KC, S], F32R, tag="xfull")
            nc.sync.dma_start(xc[:], xc_d.rearrange("(c p) t -> p c t", p=P))
            ca = attention(pool, snn, xc, w_d["cWq"], w_d["cWk"], w_d["cWv"],
                           w_d["cWo"], C_CBQ, C_CBK, C_CBO, C_CAB)
        with tc.tile_pool(name="ln2", bufs=1) as lnp:
            hT = ln(lnp, ca, C_G, C_B, dbg_name="d_h")

        # ================= FFN =================
        with tc.tile_pool(name="ffn", bufs=1) as pool:
            ut = pool.tile([P, FC, T], F32R, tag="ut")
            for m in range(FC):
                w1 = pool.tile([P, KC, P], F32R, tag="wst", bufs=3)
                nc.sync.dma_start(
                    w1[:], w1_d.rearrange("(c p) m -> p c m", p=P)
                    [:, :, m * P:(m + 1) * P])
                pu = ps.tile([P, T], F32, tag="mm", bufs=2)
                for k in range(KC):
                    nc.tensor.matmul(pu[:], w1[:, k, :], hT[:, k, :],
                                     start=(k == 0), stop=(k == KC - 1))
                nc.scalar.activation(ut[:, m, :], pu[:], AF.Relu,
                                     bias=vec[:, C_B1 + m:C_B1 + m + 1])
            if dbg:
                nc.sync.dma_start(
                    dbg_d["d_u"].rearrange("(c p) t -> p c t", p=P),
                    ut.bitcast(F32)[:])

            ff = glob.tile([P, KC, T], F32R, tag="res")
            for m in range(KC):
                w2 = pool.tile([P, FC, P], F32R, tag="w2st", bufs=2)
                nc.sync.dma_start(
                    w2[:], w2_d.rearrange("(c p) m -> p c m", p=P)
                    [:, :, m * P:(m + 1) * P])
                pf = ps.tile([P, T], F32, tag="mm", bufs=2)
                for k in range(FC):
                    nc.tensor.matmul(pf[:], w2[:, k, :], ut[:, k, :],
                                     start=(k == 0), stop=(k == FC - 1))
                nc.vector.scalar_tensor_tensor(
                    ff[:, m, :], pf[:], vec[:, C_B2 + m:C_B2 + m + 1],
                    hT.bitcast(F32)[:, m, :], op0=OP.add, op1=OP.add)

        with tc.tile_pool(name="ln3", bufs=1) as lnp:
            outT = ln(lnp, ff, C_G, C_B)
        nc.sync.dma_start(out_d.rearrange("(c p) t -> p c t", p=P),
                          outT.bitcast(F32)[:])

    _legalize_waits(nc)
    return nc


_NC_CACHE = {}


def _get_nc(dbg=False):
    if dbg not in _NC_CACHE:
        _NC_CACHE[dbg] = _build(dbg)
    return _NC_CACHE[dbg]


def _pack_chunks(v):
    """[n*128] -> [128, n] with column m = v[m*128:(m+1)*128]."""
    n = v.shape[0] // P
    return np.ascontiguousarray(v.reshape(n, P).T)


def _make_in_maps(inputs):
    hs = np.asarray(inputs["hidden_states"], np.float32)
    chs = np.asarray(inputs["cross_hidden_states"], np.float32)
    smask = np.asarray(inputs["self_att_mask"], np.float32)
    cmask = np.asarray(inputs["cross_att_mask"], np.float32)

    f32 = lambda k: np.asarray(inputs[k], np.float32)
    bos = f32("sbo") + f32("sbv") @ f32("sWo")
    boc = f32("cbo") + f32("cbv") @ f32("cWo")

    base = {n: np.ascontiguousarray(f32(n)) for n in
            ["sWq", "sWk", "sWv", "sWo", "cWq", "cWk", "cWv", "cWo"]}
    base["W1"] = np.ascontiguousarray(f32("W1"))
    base["W2"] = np.ascontiguousarray(f32("W2"))
    base["ones"] = np.ones((P, 1), np.float32)
    base["onesr"] = np.ones((1, P), np.float32)

    vec = np.zeros((P, NVEC), np.float32)
    vec[:, C_SBQ:C_SBQ + 8] = _pack_chunks(f32("sbq"))
    vec[:, C_SBK:C_SBK + 8] = _pack_chunks(f32("sbk"))
    vec[:, C_SBO:C_SBO + 8] = _pack_chunks(bos)
    vec[:, C_CBQ:C_CBQ + 8] = _pack_chunks(f32("cbq"))
    vec[:, C_CBK:C_CBK + 8] = _pack_chunks(f32("cbk"))
    vec[:, C_CBO:C_CBO + 8] = _pack_chunks(boc)
    vec[:, C_G:C_G + 8] = _pack_chunks(f32("g"))
    vec[:, C_B:C_B + 8] = _pack_chunks(f32("b"))
    vec[:, C_B1:C_B1 + 32] = _pack_chunks(f32("b1"))
    vec[:, C_B2:C_B2 + 8] = _pack_chunks(f32("b2"))
    vec[:, C_EPS] = EPS

    in_maps = []
    for c in range(8):
        b, qh = c // 2, c % 2
        qoff = qh * T
        m = dict(base)
        xkT = np.ascontiguousarray(hs[b].T)
        m["xkT"] = xkT
        m["xcT"] = np.ascontiguousarray(chs[b].T)
        m["xqT"] = np.ascontiguousarray(xkT[:, qoff:qoff + T])
        v = vec.copy()
        v[:, C_SAB:C_SAB + 8] = _pack_chunks((1.0 - smask[b]) * (-INF) / SCALE)
        v[:, C_CAB:C_CAB + 8] = _pack_chunks((1.0 - cmask[b]) * (-INF) / SCALE)
        m["vec"] = v
        in_maps.append(m)
    return in_maps


def _run(inputs, dbg=False):
    nc = _get_nc(dbg)
    in_maps = _make_in_maps(inputs)
    res = bass_utils.run_bass_kernel_spmd(nc, in_maps, core_ids=list(range(8)))
    return res.results


def kernel(**inputs) -> np.ndarray:
    results = _run(inputs, dbg=False)
    out = np.empty((B, S, H), np.float32)
    for c in range(8):
        b, qh = c // 2, c % 2
        out[b, qh * T:(qh + 1) * T, :] = results[c]["out"].T
    return out


# revision 28
# speedup vs baseline: 27488.5927x; 27488.5927x over previous
"""Trainium2 Bass kernel for nn_Block_30107720745811 (dense transformer block).

B=4, S=1024, H=1024, NH=16. 8 NeuronCores, zero-communication sharding:
core c computes batch b=c//2, query rows (c%2)*512:(c%2)*512+512.
K/V projections are duplicated within each batch pair (no collectives).

All activations live transposed [feature, token] in SBUF; weights stream
from HBM in natural [in, out] layout as matmul stationary operands.
Matmuls run in float32r (full PE rate for N>=256, ~tf32 precision).
The softmax denominator rides along the exp@V matmul as a ones column of V.
"""
import numpy as np
import concourse.bass as bass
import concourse.tile as tile
import bass_rust
from concourse import mybir
from concourse import bass_utils
from concourse.alu_op_type import AluOpType as OP

AF = mybir.ActivationFunctionType
F32 = mybir.dt.float32
F32R = mybir.dt.float32r

B, S, H, NH = 4, 1024, 1024, 16
D = H // NH          # 64
P = 128
T = 512              # query tokens per core
KC = H // P          # 8 feature chunks
FC = 4 * H // P      # 32 ffn hidden chunks
HPC = P // D         # heads per feature chunk = 2
INF = 1e10
EPS = 1e-5
SCALE = 8.0          # sqrt(D)

# vec tensor column map
C_SBQ, C_SBK, C_SBO = 0, 8, 16
C_CBQ, C_CBK, C_CBO = 24, 32, 40
C_SAB, C_CAB = 48, 56
C_G, C_B = 64, 72
C_B1, C_B2, C_EPS = 80, 112, 120
C_NW1 = 121
NVEC = 153

MAX_WAITS = 1


def _legalize_waits(nc, max_waits=MAX_WAITS):
    """Split >max_waits semaphore waits into preceding same-engine NOPs
    (this walrus build allows only one sync wait per instruction)."""
    n_split = 0
    for f in nc.m.functions:
        for blk in f.blocks:
            out = []
            for ins in blk.instructions:
                si = getattr(ins, "sync_info", None)
                if si is not None and si.on_wait and len(si.on_wait) > max_waits:
                    waits = list(si.on_wait)
                    extra, keep = waits[:-max_waits], waits[-max_waits:]
                    for j in range(0, len(extra), max_waits):
                        out.append(mybir.InstNoOp(
                            name=f"{ins.name}-lw{j}",
                            engine=ins.engine,
                            sync_info=mybir.SyncInfo(
                                on_wait=extra[j:j + max_waits], on_update=[]),
                            bass_nofuse=True,
                        ))
                    ins.sync_info = mybir.SyncInfo(
                        on_wait=keep, on_update=list(si.on_update))
                    n_split += 1
                out.append(ins)
            blk.instructions = out
    return n_split


def _build(dbg=False):
    nc = bass.Bass("TRN2", target_bir_lowering=False, debug=False,
                   dynamic_dma_scratch_size=8192)

    def din(name, shape, dt=F32R):
        return nc.dram_tensor(name, shape, dt, kind="ExternalInput").ap()

    xq_d = din("xqT", [H, T])            # query-side hidden, transposed
    xk_d = din("xkT", [H, S])            # full hidden (self K/V), transposed
    xc_d = din("xcT", [H, S])            # full cross hidden, transposed
    w_names = ["sWq", "sWk", "sWv", "sWo", "cWq", "cWk", "cWv", "cWo"]
    w_d = {n: din(n, [H, H]) for n in w_names}
    w1_d = din("W1", [H, 4 * H])
    w2_d = din("W2", [4 * H, H])
    vec_d = din("vec", [P, NVEC], F32)
    ones2_d = din("ones2", [P, P])
    out_d = nc.dram_tensor("out", [H, T], F32, kind="ExternalOutput").ap()
    dbg_d = {}
    if dbg:
        for n, shape in [("d_qT", [H, T]), ("d_kT", [H, S]), ("d_v", [S, H]),
                         ("d_at", [H, T]), ("d_sa", [H, T]), ("d_snn", [H, T]),
                         ("d_h", [H, T]), ("d_u", [4 * H, T])]:
            dbg_d[n] = nc.dram_tensor(n, shape, F32, kind="ExternalOutput").ap()

    with (
        tile.TileContext(nc) as tc,
        nc.allow_low_precision(reason="fp32r activations feed matmuls"),
        tc.tile_pool(name="glob", bufs=1) as glob,
        tc.tile_pool(name="ps", bufs=1, space="PSUM") as ps,
        tc.tile_pool(name="drs", bufs=1, space="DRAM") as drs,
    ):
        # ---- constants / vectors ----
        vec = glob.tile([P, NVEC], F32, tag="vec")
        nc.sync.dma_start(vec[:], vec_d[:])
        ones2 = glob.tile([P, P], F32R, tag="ones2")
        nc.sync.dma_start(ones2[:], ones2_d[:])
        xq = glob.tile([P, KC, T], F32R, tag="xq")

        def load_xfull(pool, src_d):
            """Load a [H, S] transposed activation in 4 chunked DMAs."""
            t = pool.tile([P, KC, S], F32R, tag="xfull")
            r = src_d.rearrange("(c p) t -> p c t", p=P)
            for j in range(4):
                nc.sync.dma_start(t[:, 2 * j:2 * j + 2, :],
                                  r[:, 2 * j:2 * j + 2, :])
            return t

        def ln_sums_start():
            psS = ps.tile([1, T], F32, tag="d", bufs=2)
            psQ = ps.tile([1, T], F32, tag="d", bufs=2)
            return psS, psQ

        def ln_sums_chunk(pool, acc, src_chunk, m):
            psS, psQ = acc
            nc.tensor.matmul(psS[:], ones2[:, 0:1], src_chunk,
                             start=(m == 0), stop=(m == KC - 1),
                             skip_group_check=True)
            sq = pool.tile([P, T], F32R, tag="sq", bufs=2)
            nc.scalar.activation(sq[:], src_chunk, AF.Square)
            nc.tensor.matmul(psQ[:], ones2[:, 0:1], sq[:],
                             start=(m == 0), stop=(m == KC - 1),
                             skip_group_check=True)

        def ln_finish(pool, acc, src, gcol, bcol, dbg_name=None, out_dma=None,
                      fused_copies=None):
            psS, psQ = acc
            mean = pool.tile([1, T], F32, tag="lnv", bufs=3)
            nc.scalar.mul(mean[:], psS[:], 1.0 / H)
            ex2 = pool.tile([1, T], F32, tag="lnv", bufs=3)
            nc.scalar.mul(ex2[:], psQ[:], 1.0 / H)
            var = pool.tile([1, T], F32, tag="lnv", bufs=3)
            nc.vector.tensor_tensor(var[:], mean[:], mean[:], op=OP.mult)
            nc.vector.tensor_tensor(var[:], ex2[:], var[:], op=OP.subtract)
            lv = pool.tile([1, T], F32, tag="lnv", bufs=3)
            nc.scalar.activation(lv[:], var[:], AF.Ln,
                                 bias=vec[0:1, C_EPS:C_EPS + 1])
            rstd = pool.tile([1, T], F32R, tag="lnr", bufs=2)
            nc.scalar.activation(rstd[:], lv[:], AF.Exp, scale=-0.5)
            meanr = pool.tile([1, T], F32R, tag="lnr", bufs=2)
            nc.vector.tensor_copy(meanr[:], mean[:])
            psA = ps.tile([P, T], F32, tag="ss", bufs=2)
            nc.tensor.matmul(psA[:], ones2[0:1, :], rstd[:], start=True,
                             stop=True)
            psC = ps.tile([P, T], F32, tag="ss", bufs=2)
            nc.tensor.matmul(psC[:], ones2[0:1, :], meanr[:], start=True,
                             stop=True)
            bcast_sb = None
            if fused_copies is not None:
                mb, ab = fused_copies
                nc.scalar.copy(mb[:], psC[:])
                nc.scalar.copy(ab[:], psA[:])
                bcast_sb = (mb, ab)
            dst = glob.tile([P, KC, T], F32R, tag="lnq")
            for m in range(KC):
                t1 = pool.tile([P, T], F32, tag="rb", bufs=2)
                nc.vector.scalar_tensor_tensor(t1[:], src.bitcast(F32)[:, m, :],
                                               0.0, psC[:], op0=OP.bypass,
                                               op1=OP.subtract)
                nc.vector.scalar_tensor_tensor(t1[:], t1[:], 0.0, psA[:],
                                               op0=OP.bypass, op1=OP.mult)
                nc.scalar.activation(dst[:, m, :], t1[:], AF.Identity,
                                     bias=vec[:, bcol + m:bcol + m + 1],
                                     scale=vec[:, gcol + m:gcol + m + 1])
                if out_dma is not None:
                    nc.sync.dma_start(out_dma[m * P:(m + 1) * P, :],
                                      dst.bitcast(F32)[:, m, :])
            if dbg and dbg_name:
                nc.sync.dma_start(
                    dbg_d[dbg_name].rearrange("(c p) t -> p c t", p=P),
                    dst.bitcast(F32)[:])
            return dst

        def attention(pool, q_src, x_kv, Wq, Wk, Wv, Wo, qb_col, kb_col,
                      ob_col, ab_col, dbg_prefix=None, post_v_hook=None,
                      ln_acc=None):
            """Full MHA incl. out-proj + residual(xq): returns sa [P, KC, T]
            f32r (glob tag 'res')."""
            # V projection, natural [token, head, dim+ones] layout
            vt = pool.tile([P, KC, NH, D + 1], F32R, tag="vt")
            for i in range(KC):
                nc.gpsimd.dma_start(vt[:, i, :, D:D + 1], ones2[:, 0:NH])
            NS = H // 4  # 256
            NHS = NS // D  # heads per slice = 4
            for n in range(4):
                wv = pool.tile([P, KC, NS], F32R, tag="wmov", bufs=2)
                nc.sync.dma_start(
                    wv[:], Wv.rearrange("(c p) n -> p c n", p=P)
                    [:, :, n * NS:(n + 1) * NS])
                if n == 0 and post_v_hook is not None:
                    post_v_hook()
                for i in range(KC):
                    pv = ps.tile([P, NS], F32, tag="mm", bufs=2)
                    for k in range(KC):
                        nc.tensor.matmul(pv[:],
                                         x_kv[:, k, i * P:(i + 1) * P],
                                         wv[:, k, :],
                                         start=(k == 0), stop=(k == KC - 1))
                    nc.vector.tensor_copy(
                        vt[:, i, n * NHS:(n + 1) * NHS, 0:D],
                        pv.rearrange("p (h d) -> p h d", d=D)[:])
            if dbg and dbg_prefix == "s":
                for i in range(KC):
                    nc.sync.dma_start(
                        dbg_d["d_v"][i * P:(i + 1) * P, :]
                        .rearrange("p (h d) -> p h d", d=D),
                        vt.bitcast(F32)[:, i, :, 0:D])

            at = pool.tile([P, KC, T], F32R, tag="at")
            wo_tiles = {}
            for mp in range(0, KC, 2):
                if mp == KC - 2:
                    wo0 = pool.tile([P, KC, 2 * P], F32R, tag="wst", bufs=3)
                    nc.sync.dma_start(
                        wo0[:], Wo.rearrange("(c p) m -> p c m", p=P)
                        [:, :, 0:2 * P])
                    wo_tiles[0] = wo0
                # paired weight loads (2 m-chunks per DMA)
                wq = pool.tile([P, KC, 2 * P], F32R, tag="wst", bufs=3)
                nc.sync.dma_start(
                    wq[:], Wq.rearrange("(c p) m -> p c m", p=P)
                    [:, :, mp * P:(mp + 2) * P])
                wk = pool.tile([P, KC, 2 * P], F32R, tag="wst", bufs=3)
                nc.sync.dma_start(
                    wk[:], Wk.rearrange("(c p) m -> p c m", p=P)
                    [:, :, mp * P:(mp + 2) * P])
                for m in (mp, mp + 1):
                    mo = (m - mp) * P
                    # Q projection chunk m
                    pq = ps.tile([P, T], F32, tag="mm", bufs=2)
                    for k in range(KC):
                        nc.tensor.matmul(pq[:], wq[:, k, mo:mo + P],
                                         q_src[:, k, :],
                                         start=(k == 0), stop=(k == KC - 1))
                    qt = pool.tile([P, T], F32R, tag="qt", bufs=2)
                    nc.scalar.activation(qt[:], pq[:], AF.Identity,
                                         bias=vec[:, qb_col + m:qb_col + m + 1])
                    if dbg and dbg_prefix == "s":
                        nc.sync.dma_start(dbg_d["d_qT"][m * P:(m + 1) * P, :],
                                          qt.bitcast(F32)[:])
                    # K projection chunk m
                    kt = pool.tile([P, S], F32R, tag="kt", bufs=2)
                    for n in range(2):
                        pk = ps.tile([P, T], F32, tag="mm", bufs=2)
                        for k in range(KC):
                            nc.tensor.matmul(pk[:], wk[:, k, mo:mo + P],
                                             x_kv[:, k, n * T:(n + 1) * T],
                                             start=(k == 0), stop=(k == KC - 1))
                        nc.scalar.activation(
                            kt[:, n * T:(n + 1) * T], pk[:], AF.Identity,
                            bias=vec[:, kb_col + m:kb_col + m + 1])
                    if dbg and dbg_prefix == "s":
                        nc.sync.dma_start(dbg_d["d_kT"][m * P:(m + 1) * P, :],
                                          kt.bitcast(F32)[:])
                    # the two heads of chunk m
                    for h2 in (1, 0):
                        h = HPC * m + h2
                        hb = h2 * D
                        psAv = ps.tile([P, T], F32, tag="av", bufs=2)
                        for i in range(KC):
                            pss = ps.tile([P, T], F32, tag="ss", bufs=2)
                            nc.tensor.matmul(pss[:],
                                             kt[hb:hb + D, i * P:(i + 1) * P],
                                             qt[hb:hb + D, :],
                                             start=True, stop=True)
                            et = pool.tile([P, T], F32R, tag="exp", bufs=3)
                            nc.scalar.activation(
                                et[:], pss[:], AF.Exp,
                                bias=vec[:, ab_col + i:ab_col + i + 1],
                                scale=1.0 / (SCALE * SCALE))
                            nc.tensor.matmul(psAv[0:D + 1, :],
                                             vt[:, i, h, :], et[:],
                                             start=(i == 0), stop=(i == KC - 1))
                        # reciprocal of denominator row (aligned at base D=64)
                        rden = pool.tile([P, T], F32R, tag="rden", bufs=1)
                        nc.vector.reciprocal(rden[D:D + 1, :], psAv[D:D + 1, :])
                        psB = ps.tile([P, T], F32, tag="av", bufs=2)
                        nc.tensor.matmul(psB[:], ones2[D:D + 1, :],
                                         rden[D:D + 1, :], start=True,
                                         stop=True)
                        rb = pool.tile([D, T], F32, tag="rb", bufs=2)
                        nc.vector.tensor_copy(rb[:], psB[0:D, :])
                        if h2 == 0:
                            nc.vector.tensor_tensor(at[0:D, m, :], psAv[0:D, :],
                                                    rb[:], op=OP.mult)
                        else:
                            atmp = pool.tile([D, T], F32R, tag="atmp", bufs=2)
                            nc.vector.tensor_tensor(atmp[:], psAv[0:D, :],
                                                    rb[:], op=OP.mult)
                            nc.sync.dma_start(at[D:P, m, :], atmp[:])
            if dbg and dbg_prefix == "s":
                nc.sync.dma_start(
                    dbg_d["d_at"].rearrange("(c p) t -> p c t", p=P),
                    at.bitcast(F32)[:])

            # out projection + bias' + residual (original xq)
            sa = glob.tile([P, KC, T], F32R, tag="res")
            for mp in range(0, KC, 2):
                if mp in wo_tiles:
                    wo = wo_tiles[mp]
                else:
                    wo = pool.tile([P, KC, 2 * P], F32R, tag="wst", bufs=3)
                    nc.sync.dma_start(
                        wo[:], Wo.rearrange("(c p) m -> p c m", p=P)
                        [:, :, mp * P:(mp + 2) * P])
                for m in (mp, mp + 1):
                    mo = (m - mp) * P
                    po = ps.tile([P, T], F32, tag="mm", bufs=2)
                    for k in range(KC):
                        nc.tensor.matmul(po[:], wo[:, k, mo:mo + P],
                                         at[:, k, :],
                                         start=(k == 0), stop=(k == KC - 1))
                    nc.vector.scalar_tensor_tensor(
                        sa[:, m, :], po[:], vec[:, ob_col + m:ob_col + m + 1],
                        xq.bitcast(F32)[:, m, :], op0=OP.add, op1=OP.add)
                    if ln_acc is not None and m > 0:
                        ln_sums_chunk(pool, ln_acc, sa[:, m - 1, :], m - 1)
            if ln_acc is not None:
                ln_sums_chunk(pool, ln_acc, sa[:, KC - 1, :], KC - 1)
            return sa

        # ====== self attention + LN1 + cross attention + LN2 (one pool) =====
        with tc.tile_pool(name="attn", bufs=1) as pool:
            xk = load_xfull(pool, xk_d)

            def _load_xq():
                nc.sync.dma_start(
                    xq[:], xq_d.rearrange("(c p) t -> p c t", p=P))

            acc1 = ln_sums_start()
            sa = attention(pool, xq, xk, w_d["sWq"], w_d["sWk"], w_d["sWv"],
                           w_d["sWo"], C_SBQ, C_SBK, C_SBO, C_SAB,
                           dbg_prefix="s", post_v_hook=_load_xq, ln_acc=acc1)
            if dbg:
                nc.sync.dma_start(
                    dbg_d["d_sa"].rearrange("(c p) t -> p c t", p=P),
                    sa.bitcast(F32)[:])
            snn = ln_finish(pool, acc1, sa, C_G, C_B, dbg_name="d_snn")
            xc = load_xfull(pool, xc_d)
            acc2 = ln_sums_start()
            ca = attention(pool, snn, xc, w_d["cWq"], w_d["cWk"], w_d["cWv"],
                           w_d["cWo"], C_CBQ, C_CBK, C_CBO, C_CAB,
                           ln_acc=acc2)

        # ================= FFN (LN2 inside, weights prefetched) ============
        with tc.tile_pool(name="ffn", bufs=1) as pool:
            w1r = w1_d.rearrange("(c p) m -> p c m", p=P)
            w2r = w2_d.rearrange("(c p) m -> p c m", p=P)
            w1_tiles = {}
            w1f = pool.tile([P, KC, P], F32R, tag="w1f", bufs=1)
            nc.sync.dma_start(w1f[:], w1r[:, :, 0:P])
            w1_tiles["f"] = w1f
            w1 = pool.tile([P, KC, 3 * P], F32R, tag="wst", bufs=2)
            nc.sync.dma_start(w1[:], w1r[:, :, P:4 * P])
            w1_tiles[0] = w1
            w1 = pool.tile([P, KC, 4 * P], F32R, tag="wst", bufs=2)
            nc.sync.dma_start(w1[:], w1r[:, :, 4 * P:8 * P])
            w1_tiles[4] = w1
            w2_tiles = {}
            for m0 in (0, 1):
                w2 = pool.tile([P, FC, P], F32R, tag="w2st", bufs=2)
                nc.sync.dma_start(w2[:], w2r[:, :, m0 * P:(m0 + 1) * P])
                w2_tiles[m0] = w2

            mb = pool.tile([P, T], F32, tag="lnb", bufs=2)
            ab = pool.tile([P, T], F32, tag="lnb", bufs=2)
            hT = ln_finish(pool, acc2, ca, C_G, C_B, dbg_name="d_h",
                           fused_copies=(mb, ab))

            # FFN1 consumes pre-LN ca directly; the LN correction commutes
            # through the contraction: u = relu((W1^T ca - colsum(W1) mean)
            # * rstd + b1)
            ut = pool.tile([P, FC, T], F32R, tag="ut")
            for mp in range(0, FC, 4):
                if mp in w1_tiles:
                    w1 = w1_tiles[mp]
                    moff = P if mp == 0 else 0
                elif mp == 4:
                    w1 = w1_tiles[4]
                    moff = -4 * P
                else:
                    w1 = pool.tile([P, KC, 4 * P], F32R, tag="wst", bufs=2)
                    nc.sync.dma_start(w1[:], w1r[:, :, mp * P:(mp + 4) * P])
                    moff = 0
                for m in range(mp, mp + 4):
                    if mp == 0 and m == 0:
                        w1u, mo = w1_tiles["f"], 0
                    elif mp == 0:
                        w1u, mo = w1, (m - 1) * P
                    else:
                        w1u, mo = w1, (m - mp) * P + moff
                    pu = ps.tile([P, T], F32, tag="mm", bufs=2)
                    for k in range(KC):
                        nc.tensor.matmul(pu[:], w1u[:, k, mo:mo + P],
                                         ca[:, k, :],
                                         start=(k == 0), stop=(k == KC - 1))
                    t1 = pool.tile([P, T], F32, tag="rb", bufs=2)
                    nc.vector.scalar_tensor_tensor(
                        t1[:], mb[:], vec[:, C_NW1 + m:C_NW1 + m + 1], pu[:],
                        op0=OP.mult, op1=OP.add)
                    nc.vector.tensor_tensor(t1[:], t1[:], ab[:], op=OP.mult)
                    nc.scalar.activation(ut[:, m, :], t1[:], AF.Relu,
                                         bias=vec[:, C_B1 + m:C_B1 + m + 1])
            if dbg:
                nc.sync.dma_start(
                    dbg_d["d_u"].rearrange("(c p) t -> p c t", p=P),
                    ut.bitcast(F32)[:])

            ff = glob.tile([P, KC, T], F32R, tag="res")
            acc3 = ln_sums_start()
            for m in range(KC):
                if m in w2_tiles:
                    w2 = w2_tiles[m]
                else:
                    w2 = pool.tile([P, FC, P], F32R, tag="w2st", bufs=2)
                    nc.sync.dma_start(w2[:], w2r[:, :, m * P:(m + 1) * P])
                pf = ps.tile([P, T], F32, tag="mm", bufs=2)
                for k in range(FC):
                    nc.tensor.matmul(pf[:], w2[:, k, :], ut[:, k, :],
                                     start=(k == 0), stop=(k == FC - 1))
                nc.vector.scalar_tensor_tensor(
                    ff[:, m, :], pf[:], vec[:, C_B2 + m:C_B2 + m + 1],
                    hT.bitcast(F32)[:, m, :], op0=OP.add, op1=OP.add)
                if m > 0:
                    ln_sums_chunk(pool, acc3, ff[:, m - 1, :], m - 1)
            ln_sums_chunk(pool, acc3, ff[:, KC - 1, :], KC - 1)

        with tc.tile_pool(name="ln3", bufs=1) as pool:
            ln_finish(pool, acc3, ff, C_G, C_B, out_dma=out_d)

    _legalize_waits(nc)
    return nc


_NC_CACHE = {}


def _get_nc(dbg=False):
    if dbg not in _NC_CACHE:
        _NC_CACHE[dbg] = _build(dbg)
    return _NC_CACHE[dbg]


def _pack_chunks(v):
    """[n*128] -> [128, n] with column m = v[m*128:(m+1)*128]."""
    n = v.shape[0] // P
    return np.ascontiguousarray(v.reshape(n, P).T)


def _make_in_maps(inputs):
    hs = np.asarray(inputs["hidden_states"], np.float32)
    chs = np.asarray(inputs["cross_hidden_states"], np.float32)
    smask = np.asarray(inputs["self_att_mask"], np.float32)
    cmask = np.asarray(inputs["cross_att_mask"], np.float32)

    f32 = lambda k: np.asarray(inputs[k], np.float32)
    bos = f32("sbo") + f32("sbv") @ f32("sWo")
    boc = f32("cbo") + f32("cbv") @ f32("cWo")

    base = {n: np.ascontiguousarray(f32(n)) for n in
            ["sWq", "sWk", "sWv", "sWo", "cWq", "cWk", "cWv", "cWo"]}
    base["W1"] = np.ascontiguousarray(f32("W1"))
    base["W2"] = np.ascontiguousarray(f32("W2"))
    base["ones2"] = np.ones((P, P), np.float32)

    vec = np.zeros((P, NVEC), np.float32)
    vec[:, C_SBQ:C_SBQ + 8] = _pack_chunks(f32("sbq"))
    vec[:, C_SBK:C_SBK + 8] = _pack_chunks(f32("sbk"))
    vec[:, C_SBO:C_SBO + 8] = _pack_chunks(bos)
    vec[:, C_CBQ:C_CBQ + 8] = _pack_chunks(f32("cbq"))
    vec[:, C_CBK:C_CBK + 8] = _pack_chunks(f32("cbk"))
    vec[:, C_CBO:C_CBO + 8] = _pack_chunks(boc)
    vec[:, C_G:C_G + 8] = _pack_chunks(f32("g"))
    vec[:, C_B:C_B + 8] = _pack_chunks(f32("b"))
    vec[:, C_B1:C_B1 + 32] = _pack_chunks(f32("b1"))
    vec[:, C_B2:C_B2 + 8] = _pack_chunks(f32("b2"))
    vec[:, C_NW1:C_NW1 + 32] = _pack_chunks(-f32("W1").sum(axis=0))
    vec[:, C_EPS] = EPS

    in_maps = []
    for c in range(8):
        b, qh = c // 2, c % 2
        qoff = qh * T
        m = dict(base)
        xkT = np.ascontiguousarray(hs[b].T)
        m["xkT"] = xkT
        m["xcT"] = np.ascontiguousarray(chs[b].T)
        m["xqT"] = np.ascontiguousarray(xkT[:, qoff:qoff + T])
        v = vec.copy()
        v[:, C_SAB:C_SAB + 8] = _pack_chunks((1.0 - smask[b]) * (-INF) / SCALE)
        v[:, C_CAB:C_CAB + 8] = _pack_chunks((1.0 - cmask[b]) * (-INF) / SCALE)
        m["vec"] = v
        in_maps.append(m)
    return in_maps


def _run(inputs, dbg=False):
    nc = _get_nc(dbg)
    in_maps = _make_in_maps(inputs)
    res = bass_utils.run_bass_kernel_spmd(nc, in_maps, core_ids=list(range(8)))
    return res.results


def kernel(**inputs) -> np.ndarray:
    results = _run(inputs, dbg=False)
    out = np.empty((B, S, H), np.float32)
    for c in range(8):
        b, qh = c // 2, c % 2
        out[b, qh * T:(qh + 1) * T, :] = results[c]["out"].T
    return out
